# revision 15
# baseline (speedup 1.0000x reference)
"""Trainium2 Bass kernel for nn_Block_56968446214461 (GNN message passing block).

Data parallel over batch: B=4096 split across 8 NeuronCores (512 each).
Per-core tiling: 74 "adjacency tiles" of 7 batch elements (119 tokens, last
tile overlap-reads and writes only the remainder).

Layouts:
  T  (token-major):   [tokens(P), channels(free)]
  F  (feature-major): [channels(P: 4 chunks of 128), tokens(free)]
Channel matmuls run F->T (stationary = activation^T chunk, moving = W^T
slices, N=512 -> float32r at 1 cyc/col). Adjacency contraction runs T->T with
a block-diagonal adjacency as stationary.

Optimizations over the serial baseline (5.22ms -> 3.17ms):
  - Softmax denominators broadcast on-chip: reciprocal of the fused
    ones-column row, then a [1,64]x[1,4*TOK] ones-stationary matmul into
    PSUM, landed in SBUF for the per-head divides. (The baseline round-
    tripped the row through DRAM, idling all engines ~16us per tile and
    letting the PE de-ramp.)
  - Activation-table steering: Ln is dropped from natural_log and Exp from
    exp_and_others in the (process-cached) act-table dict, so the load
    inserter picks natural_log_exp_and_others for both; only the MLP Gelu
    block switches tables (2 loads/tile instead of 8, saving ~7.7us/tile of
    Activation-engine time). Set contents only shrink, so every emitted
    table id remains valid for the functions run under it.
  - Attention in bf16 (q,k via bf16 PE transposes into bitcast PSUM, exp->
    bf16 U, bf16 v/mask): the f32 small-free-dim matmuls (4 cyc/col) drop
    to 1 cyc/col.
  - Elementwise ops spread across DVE / Act / Pool(gpsimd); transpose
    drains batched per 4-chunk group; per-k GCN intermediates in a
    3-deep rotation so adjacent stages overlap.
"""

import numpy as np

import concourse.bacc as bacc
from concourse.hw_specs import get_activation_tables
import concourse.bass as bass
import concourse.tile as tile
from concourse import mybir
from concourse import bass_utils

f32 = mybir.dt.float32
f32r = mybir.dt.float32r
bf16 = mybir.dt.bfloat16

B, J, C = 4096, 17, 512
H, D, K = 8, 64, 3
N_CORES = 8
B_CORE = B // N_CORES
NB = 7
TOK = NB * J  # 119
CK = C // 128  # 4 cin chunks
EPS = 1e-5

_CACHE = {}


def _tiles(b_core):
    out = []
    i = 0
    while (i + 1) * NB <= b_core:
        out.append((i * NB, i * NB, NB))
        i += 1
    rem = b_core - i * NB
    if rem:
        out.append((b_core - NB, b_core - rem, rem))
    return out


def _bcast_row_ap(t_ap, offset_elems, nparts, n):
    """AP reading one sbuf row (partition fixed) broadcast to nparts partitions."""
    return bass.AP(
        tensor=t_ap.tensor,
        offset=t_ap.offset + offset_elems,
        ap=[[0, nparts], [1, n]],
    )


def _nle_first(arch):
    """Steer the act-table chooser to natural_log_exp_and_others for Ln AND
    Exp without disturbing table ids (walrus maps ids by act_info.json file
    order): drop Ln from natural_log and Exp from exp_and_others in the
    cached dict. The mutated sets are subsets of the real ones, so every
    emitted load id stays valid for the functions run under it."""
    tabs = get_activation_tables(arch)
    if "natural_log_exp_and_others" not in tabs:
        return
    tabs.get("natural_log", set()).discard(mybir.ActivationFunctionType.Ln)
    tabs.get("exp_and_others", set()).discard(mybir.ActivationFunctionType.Exp)


def _build(b_core, flags):
    ln1aff = "ln1aff" in flags
    bias_on = {k for k in flags if k.startswith("b_")}

    nc = bacc.Bacc("TRN2", target_bir_lowering=False, debug=False)
    _nle_first(nc.m.arch)
    ntok = b_core * J

    # DRAM I/O
    x2d = nc.dram_tensor("x2d", [ntok, C], f32, kind="ExternalInput")
    xTd = nc.dram_tensor("xTd", [C, ntok], f32r, kind="ExternalInput")
    w1T = nc.dram_tensor("w1T", [C, K * C], f32r, kind="ExternalInput")
    wqkvT = nc.dram_tensor("wqkvT", [C, 3 * C], f32r, kind="ExternalInput")
    wpT = nc.dram_tensor("wpT", [C, C], f32r, kind="ExternalInput")
    w2T = nc.dram_tensor("w2T", [C, K * C], f32r, kind="ExternalInput")
    m1T = nc.dram_tensor("m1T", [C, 256], f32r, kind="ExternalInput")
    m2T = nc.dram_tensor("m2T", [256, 256], f32r, kind="ExternalInput")
    m3T = nc.dram_tensor("m3T", [256, C], f32r, kind="ExternalInput")
    ablkd = nc.dram_tensor("ablk", [TOK, K * TOK], f32r, kind="ExternalInput")
    maskd = nc.dram_tensor("maskd", [TOK, TOK], f32r, kind="ExternalInput")
    identd = nc.dram_tensor("identd", [TOK, TOK], f32, kind="ExternalInput")
    g1d = nc.dram_tensor("g1d", [J], f32, kind="ExternalInput")
    b1d = nc.dram_tensor("b1d", [J], f32, kind="ExternalInput")
    biasd = {}
    for nm, ln in [("b_qkv", 3 * C), ("b_p", C), ("b_1g", K * C), ("b_2g", K * C),
                   ("b_m1", 256), ("b_m2", 256), ("b_m3", C)]:
        if nm in bias_on:
            biasd[nm] = nc.dram_tensor(nm, [ln], f32, kind="ExternalInput")
    outd = nc.dram_tensor("out", [ntok, C], f32, kind="ExternalOutput")

    with tile.TileContext(nc) as tc:
        with tc.tile_pool(name="const", bufs=1) as cpool, \
             tc.tile_pool(name="act", bufs=2) as apool, \
             tc.tile_pool(name="scr", bufs=2) as spool, \
             tc.tile_pool(name="act1", bufs=6) as a1pool, \
             tc.tile_pool(name="ps_y", bufs=3, space="PSUM") as ps_y, \
             tc.tile_pool(name="ps_one", bufs=2, space="PSUM") as ps_one, \
             tc.tile_pool(name="ps_tr", bufs=1, space="PSUM") as ps_tr, \
             tc.tile_pool(name="ps_sc", bufs=1, space="PSUM") as ps_sc, \
             tc.tile_pool(name="ps_oz", bufs=1, space="PSUM") as ps_oz:

            # ---- one-time weight / constant loads ----
            w1s = cpool.tile([128, CK, K * C], f32r)
            nc.sync.dma_start(out=w1s, in_=w1T.ap().rearrange("(c p) n -> p c n", c=CK))
            wqs = cpool.tile([128, CK, 3 * C], f32r)
            nc.sync.dma_start(out=wqs, in_=wqkvT.ap().rearrange("(c p) n -> p c n", c=CK))
            wps = cpool.tile([128, CK, C], f32r)
            nc.sync.dma_start(out=wps, in_=wpT.ap().rearrange("(c p) n -> p c n", c=CK))
            w2s = cpool.tile([128, CK, K * C], f32r)
            nc.sync.dma_start(out=w2s, in_=w2T.ap().rearrange("(c p) n -> p c n", c=CK))
            m1s = cpool.tile([128, CK, 256], f32r)
            nc.sync.dma_start(out=m1s, in_=m1T.ap().rearrange("(c p) n -> p c n", c=CK))
            m2s = cpool.tile([128, 2, 256], f32r)
            nc.sync.dma_start(out=m2s, in_=m2T.ap().rearrange("(c p) n -> p c n", c=2))
            m3s = cpool.tile([128, 2, C], f32r)
            nc.sync.dma_start(out=m3s, in_=m3T.ap().rearrange("(c p) n -> p c n", c=2))
            ablk = cpool.tile([TOK, K, TOK], f32r)
            nc.sync.dma_start(out=ablk, in_=ablkd.ap().rearrange("p (k w) -> p k w", k=K))
            mask = cpool.tile([TOK, TOK], f32r)
            nc.sync.dma_start(out=mask, in_=maskd.ap())
            ident = cpool.tile([TOK, TOK], f32)
            nc.sync.dma_start(out=ident, in_=identd.ap())
            maskb = cpool.tile([TOK, TOK], bf16)
            nc.vector.tensor_copy(out=maskb, in_=mask)
            identb = cpool.tile([TOK, TOK], bf16)
            nc.vector.tensor_copy(out=identb, in_=ident)
            epst = cpool.tile([128, 1], f32)
            nc.vector.memset(epst, EPS)
            ones64f = cpool.tile([1, 64], f32)
            nc.vector.memset(ones64f, 1.0)
            ones64 = ones64f.bitcast(f32r)
            if ln1aff:
                g1t = cpool.tile([128, J], f32)
                nc.sync.dma_start(out=g1t, in_=_bcast_row_ap(g1d.ap(), 0, 128, J))
                b1t = cpool.tile([128, J], f32)
                nc.sync.dma_start(out=b1t, in_=_bcast_row_ap(b1d.ap(), 0, 128, J))
            btiles = {}
            for nm, t in biasd.items():
                ln = t.shape[1] if len(t.shape) > 1 else t.shape[0]
                bt = cpool.tile([128, ln], f32, tag=f"bt_{nm}")
                nc.sync.dma_start(out=bt, in_=_bcast_row_ap(t.ap(), 0, 128, ln))
                btiles[nm] = bt

            x2a = x2d.ap()
            xTa = xTd.ap().rearrange("(c p) t -> p c t", c=CK)
            outa = outd.ap()

            for (b0, wb0, wnb) in _tiles(b_core):
                t0 = b0 * J
                woff = (wb0 - b0) * J
                wntok = wnb * J

                # ---- loads ----
                xT = apool.tile([TOK, C], f32, tag="xT")
                nc.sync.dma_start(out=xT, in_=x2a[t0:t0 + TOK, :])
                xF = apool.tile([128, CK, TOK], f32r, tag="xF")
                nc.scalar.dma_start(out=xF, in_=xTa[:, :, t0:t0 + TOK])

                # ---- LN1 over joints (F layout; j innermost) ----
                xFg = xF.rearrange("p c (b j) -> p c b j", j=J)
                s1 = spool.tile([128, CK, NB], f32, tag="s1")
                nc.vector.tensor_reduce(out=s1, in_=xFg, axis=mybir.AxisListType.X,
                                        op=mybir.AluOpType.add)
                xsq = spool.tile([128, CK, TOK], f32, tag="xsq")
                nc.gpsimd.tensor_mul(out=xsq, in0=xF, in1=xF)
                s2 = spool.tile([128, CK, NB], f32, tag="s2")
                nc.vector.tensor_reduce(out=s2,
                                        in_=xsq.rearrange("p c (b j) -> p c b j", j=J),
                                        axis=mybir.AxisListType.X,
                                        op=mybir.AluOpType.add)
                mj = spool.tile([128, CK, NB], f32, tag="mj")
                nc.scalar.mul(out=mj, in_=s1, mul=1.0 / J)
                msq = spool.tile([128, CK, NB], f32, tag="msq")
                nc.gpsimd.tensor_mul(out=msq, in0=mj, in1=mj)
                varj = spool.tile([128, CK, NB], f32, tag="varj")
                nc.vector.scalar_tensor_tensor(out=varj, in0=s2, scalar=1.0 / J,
                                               in1=msq, op0=mybir.AluOpType.mult,
                                               op1=mybir.AluOpType.subtract)
                sdj = spool.tile([128, CK, NB], f32, tag="sdj")
                nc.scalar.activation(out=sdj, in_=varj,
                                     func=mybir.ActivationFunctionType.Ln,
                                     bias=epst, scale=1.0)
                rj = spool.tile([128, CK, NB], f32, tag="rj")
                nc.scalar.activation(out=rj, in_=sdj,
                                     func=mybir.ActivationFunctionType.Exp,
                                     scale=-0.5)
                mrj = spool.tile([128, CK, NB], f32, tag="mrj")
                nc.gpsimd.tensor_mul(out=mrj, in0=mj, in1=rj)
                xg = apool.tile([128, CK, TOK], f32r, tag="xg")
                xgg = xg.rearrange("p c (b j) -> p c b j", j=J)
                tmp1 = spool.tile([128, CK, TOK], f32, tag="tmp1")
                t1g = tmp1.rearrange("p c (b j) -> p c b j", j=J)
                nc.gpsimd.tensor_mul(out=t1g, in0=xFg,
                                     in1=rj.to_broadcast([128, CK, NB, J]))
                nc.gpsimd.tensor_sub(out=xgg, in0=t1g,
                                     in1=mrj.to_broadcast([128, CK, NB, J]))
                if ln1aff:
                    ga = g1t
                    gb = bass.AP(tensor=ga.tensor, offset=ga.offset,
                                 ap=[ga.ap[0], [0, CK], [0, NB], ga.ap[1]])
                    ba = b1t
                    bb = bass.AP(tensor=ba.tensor, offset=ba.offset,
                                 ap=[ba.ap[0], [0, CK], [0, NB], ba.ap[1]])
                    nc.vector.tensor_mul(out=xgg, in0=xgg, in1=gb)
                    nc.vector.tensor_add(out=xgg, in0=xgg, in1=bb)

                # ---- GCN1: per-k matmul -> drain -> adjacency accumulate ----
                xg1p = ps_one.tile([TOK, C], f32, tag="one")
                for k in range(K):
                    y1p = ps_y.tile([TOK, C], f32, tag="y", name="y1p")
                    for c in range(CK):
                        nc.tensor.matmul(y1p, xg[:, c, :],
                                         w1s[:, c, k * C:(k + 1) * C],
                                         start=(c == 0), stop=(c == CK - 1))
                    yk = a1pool.tile([TOK, C], f32r, tag="yk", name="yk1")
                    if "b_1g" in bias_on:
                        nc.vector.tensor_add(out=yk, in0=y1p,
                                             in1=btiles["b_1g"][:TOK, k * C:(k + 1) * C])
                    elif k == 1:
                        nc.scalar.copy(out=yk, in_=y1p)
                    else:
                        nc.vector.tensor_copy(out=yk, in_=y1p)
                    nc.tensor.matmul(xg1p, ablk[:, k, :], yk,
                                     start=(k == 0), stop=(k == K - 1))
                xg1 = apool.tile([TOK, C], f32, tag="xg1")
                nc.scalar.copy(out=xg1, in_=xg1p)
                # transpose -> xg1F
                xg1F = apool.tile([128, CK, TOK], f32, tag="xg1F")
                tp = ps_tr.tile([128, 4, TOK], f32, tag="tr")
                for c in range(CK):
                    nc.tensor.transpose(tp[:, c, :], xg1[:, c * 128:(c + 1) * 128], ident)
                nc.vector.tensor_copy(out=xg1F, in_=tp)

                # ---- lnA (over channels, T layout) + transpose xa -> xaF ----
                st = spool.tile([TOK, 6], f32, tag="st")
                nc.vector.bn_stats(out=st, in_=xT)
                mv = spool.tile([TOK, 2], f32, tag="mv")
                nc.vector.bn_aggr(out=mv, in_=st)
                sda = spool.tile([TOK, 1], f32, tag="sda")
                nc.scalar.activation(out=sda, in_=mv[:, 1:2],
                                     func=mybir.ActivationFunctionType.Ln,
                                     bias=epst[:TOK], scale=1.0)
                ra = spool.tile([TOK, 1], f32, tag="ra")
                nc.scalar.activation(out=ra, in_=sda,
                                     func=mybir.ActivationFunctionType.Exp,
                                     scale=-0.5)
                xa = apool.tile([TOK, C], f32, tag="xa")
                nc.vector.tensor_scalar(out=xa, in0=xT, scalar1=mv[:, 0:1],
                                        scalar2=ra, op0=mybir.AluOpType.subtract,
                                        op1=mybir.AluOpType.mult)
                xaF = apool.tile([128, CK, TOK], f32r, tag="xaF")
                tp = ps_tr.tile([128, 4, TOK], f32, tag="tr")
                for c in range(CK):
                    nc.tensor.transpose(tp[:, c, :], xa[:, c * 128:(c + 1) * 128], ident)
                nc.scalar.copy(out=xaF, in_=tp)

                # ---- qkv matmul (F->T): per-s [tok, 512] ----
                qkT = apool.tile([TOK, 2, C], bf16, tag="qkT")
                for s in range(2):
                    qp = ps_y.tile([TOK, C], f32, tag="y", name="qp")
                    for c in range(CK):
                        nc.tensor.matmul(qp, xaF[:, c, :],
                                         wqs[:, c, s * C:(s + 1) * C],
                                         start=(c == 0), stop=(c == CK - 1))
                    if "b_qkv" in bias_on:
                        nc.vector.tensor_add(out=qkT[:, s, :], in0=qp,
                                             in1=btiles["b_qkv"][:TOK, s * C:(s + 1) * C])
                    elif s == 1:
                        nc.scalar.copy(out=qkT[:, s, :], in_=qp)
                    else:
                        nc.vector.tensor_copy(out=qkT[:, s, :], in_=qp)
                qkF = apool.tile([128, 8, TOK], bf16, tag="qkF")
                qkT2 = qkT.rearrange("p s c -> p (s c)")
                for g in range(2):
                    tp = ps_tr.tile([128, 4, TOK], f32, tag="tr")
                    tpb = tp.bitcast(bf16)
                    for cc in range(4):
                        c = g * 4 + cc
                        nc.tensor.transpose(tpb[:, cc, 0:TOK], qkT2[:, c * 128:(c + 1) * 128], identb)
                    if g == 0:
                        nc.vector.tensor_copy(out=qkF[:, 0:4, :], in_=tpb[:, :, 0:TOK])
                    else:
                        nc.scalar.copy(out=qkF[:, 4:8, :], in_=tpb[:, :, 0:TOK])
                # v -> sbuf with per-head stride 65 (col 64 = ones)
                vp = ps_y.tile([TOK, C], f32, tag="y", name="vp")
                for c in range(CK):
                    nc.tensor.matmul(vp, xaF[:, c, :],
                                     wqs[:, c, 2 * C:3 * C],
                                     start=(c == 0), stop=(c == CK - 1))
                vsb = apool.tile([TOK, H, 65], bf16, tag="vsb")
                nc.gpsimd.memset(vsb[:, :, 64:65], 1.0)
                vdst = vsb[:, :, 0:64]
                vsrc = vp.rearrange("p (h d) -> p h d", h=H)
                if "b_qkv" in bias_on:
                    bq = btiles["b_qkv"][:TOK, 2 * C:3 * C] \
                        .rearrange("p (h d) -> p h d", h=H)
                    nc.vector.tensor_add(out=vdst, in0=vsrc, in1=bq)
                else:
                    nc.vector.tensor_copy(out=vdst, in_=vsrc)

                # ---- attention (two half-passes of 4 heads) ----
                rzs = spool.tile([1, H, TOK], f32r, tag="rzs")
                oF = apool.tile([128, CK, TOK], f32, tag="oF")
                for half in range(2):
                    scp = ps_sc.tile([TOK, 4, TOK], f32, tag="sc")
                    ozp = ps_oz.tile([65, 4, TOK], f32, tag="oz")
                    U = apool.tile([TOK, 4, TOK], bf16, tag="U")
                    for hh in range(4):
                        h = half * 4 + hh
                        kap = qkF[(h % 2) * 64:(h % 2) * 64 + 64, 4 + h // 2, :]
                        qap = qkF[(h % 2) * 64:(h % 2) * 64 + 64, h // 2, :]
                        nc.tensor.matmul(scp[:, hh, :], kap, qap,
                                         start=True, stop=True)
                        nc.scalar.activation(out=U[:, hh, :], in_=scp[:, hh, :],
                                             func=mybir.ActivationFunctionType.Exp,
                                             scale=float(D) ** -0.5)
                    ma = maskb
                    mb = bass.AP(tensor=ma.tensor, offset=ma.offset,
                                 ap=[ma.ap[0], [0, 4], ma.ap[1]])
                    nc.vector.tensor_mul(out=U, in0=U, in1=mb)
                    for hh in range(4):
                        h = half * 4 + hh
                        nc.tensor.matmul(ozp[:, hh, :], vsb[:, h, :], U[:, hh, :],
                                         start=True, stop=True)
                    with nc.allow_low_precision(reason="recip row feeds bcast matmul"):
                        nc.vector.reciprocal(out=rzs[:, half * 4:half * 4 + 4, :],
                                             in_=ozp[64:65, :, :])
                    rbp = ps_tr.tile([128, 4, TOK], f32, tag="tr")
                    nc.tensor.matmul(rbp[0:64].rearrange("p h t -> p (h t)"), ones64,
                                     rzs[0:1, half * 4:half * 4 + 4, :]
                                     .rearrange("p h t -> p (h t)"),
                                     start=True, stop=True)
                    rbs = spool.tile([64, 4, TOK], f32, tag="rbs")
                    nc.vector.tensor_copy(out=rbs, in_=rbp[0:64])
                    nc.vector.tensor_mul(
                        out=oF[0:64, 2 * half:2 * half + 2, :],
                        in0=ozp[0:64, 0::2, :], in1=rbs[:, 0::2, :])
                    nc.vector.tensor_mul(
                        out=oF[64:128, 2 * half:2 * half + 2, :],
                        in0=ozp[0:64, 1::2, :], in1=rbs[:, 1::2, :])

                # ---- proj input / gcn2 input (F) ----
                # per-half so proj/gcn2 start on chunks 0-1 while attention
                # half 1 still runs (oF chunks 2h..2h+1 come from half h)
                pin = apool.tile([128, CK, TOK], f32r, tag="pin")
                gin = apool.tile([128, CK, TOK], f32r, tag="gin")
                for hf in (slice(0, 2), slice(2, 4)):
                    nc.vector.scalar_tensor_tensor(
                        out=pin[:, hf, :], in0=xg1F[:, hf, :], scalar=0.5,
                        in1=oF[:, hf, :], op0=mybir.AluOpType.mult,
                        op1=mybir.AluOpType.add)
                    nc.vector.scalar_tensor_tensor(
                        out=gin[:, hf, :], in0=oF[:, hf, :], scalar=0.8,
                        in1=xg1F[:, hf, :], op0=mybir.AluOpType.mult,
                        op1=mybir.AluOpType.add)

                # ---- proj matmul ----
                xap = ps_one.tile([TOK, C], f32, tag="one")
                for c in range(CK):
                    nc.tensor.matmul(xap, pin[:, c, :], wps[:, c, :],
                                     start=(c == 0), stop=(c == CK - 1))

                # ---- gcn2: per-k matmul -> drain -> adjacency accumulate ----
                xg2p = ps_one.tile([TOK, C], f32, tag="one")
                for k in range(K):
                    y2p = ps_y.tile([TOK, C], f32, tag="y", name="y2p")
                    for c in range(CK):
                        nc.tensor.matmul(y2p, gin[:, c, :],
                                         w2s[:, c, k * C:(k + 1) * C],
                                         start=(c == 0), stop=(c == CK - 1))
                    yk = a1pool.tile([TOK, C], f32r, tag="yk", name="yk2")
                    if "b_2g" in bias_on:
                        nc.vector.tensor_add(out=yk, in0=y2p,
                                             in1=btiles["b_2g"][:TOK, k * C:(k + 1) * C])
                    elif k == 1:
                        nc.scalar.copy(out=yk, in_=y2p)
                    else:
                        nc.vector.tensor_copy(out=yk, in_=y2p)
                    nc.tensor.matmul(xg2p, ablk[:, k, :], yk,
                                     start=(k == 0), stop=(k == K - 1))

                # ---- y = x + xg2 + xa_out ----
                yT = apool.tile([TOK, C], f32, tag="yT")
                nc.vector.tensor_add(out=yT, in0=xap, in1=xT)
                if "b_p" in bias_on:
                    nc.vector.tensor_add(out=yT, in0=yT,
                                         in1=btiles["b_p"][:TOK, :])
                nc.vector.tensor_add(out=yT, in0=xg2p, in1=yT)

                # ---- LN2 + transpose z ----
                st2 = spool.tile([TOK, 6], f32, tag="st2")
                nc.vector.bn_stats(out=st2, in_=yT)
                mv2 = spool.tile([TOK, 2], f32, tag="mv2")
                nc.vector.bn_aggr(out=mv2, in_=st2)
                sd2 = spool.tile([TOK, 1], f32, tag="sd2")
                nc.scalar.activation(out=sd2, in_=mv2[:, 1:2],
                                     func=mybir.ActivationFunctionType.Ln,
                                     bias=epst[:TOK], scale=1.0)
                r2 = spool.tile([TOK, 1], f32, tag="r2")
                nc.scalar.activation(out=r2, in_=sd2,
                                     func=mybir.ActivationFunctionType.Exp,
                                     scale=-0.5)
                z = apool.tile([TOK, C], f32, tag="z")
                nc.vector.tensor_scalar(out=z, in0=yT, scalar1=mv2[:, 0:1],
                                        scalar2=r2, op0=mybir.AluOpType.subtract,
                                        op1=mybir.AluOpType.mult)
                zF = apool.tile([128, CK, TOK], f32r, tag="zF")
                tp = ps_tr.tile([128, 4, TOK], f32, tag="tr")
                for c in range(CK):
                    nc.tensor.transpose(tp[:, c, :], z[:, c * 128:(c + 1) * 128], ident)
                nc.vector.tensor_copy(out=zF, in_=tp)

                # ---- MLP ----
                h1p = ps_one.tile([TOK, C], f32, tag="one")
                for c in range(CK):
                    nc.tensor.matmul(h1p[:, 0:256], zF[:, c, :], m1s[:, c, :],
                                     start=(c == 0), stop=(c == CK - 1))
                h1 = apool.tile([TOK, 256], f32, tag="h1")
                if "b_m1" in bias_on:
                    tb1 = spool.tile([TOK, 256], f32, tag="tb1")
                    nc.vector.tensor_add(out=tb1, in0=h1p[:, 0:256],
                                         in1=btiles["b_m1"][:TOK, :])
                    nc.scalar.activation(out=h1, in_=tb1,
                                         func=mybir.ActivationFunctionType.Gelu)
                else:
                    nc.scalar.activation(out=h1, in_=h1p[:, 0:256],
                                         func=mybir.ActivationFunctionType.Gelu)
                h1F = apool.tile([128, 2, TOK], f32r, tag="h1F")
                tp = ps_tr.tile([128, 4, TOK], f32, tag="tr")
                for c in range(2):
                    nc.tensor.transpose(tp[:, c, :], h1[:, c * 128:(c + 1) * 128], ident)
                nc.scalar.copy(out=h1F, in_=tp[:, 0:2, :])

                h2p = ps_one.tile([TOK, C], f32, tag="one")
                for c in range(2):
                    nc.tensor.matmul(h2p[:, 0:256], h1F[:, c, :], m2s[:, c, :],
                                     start=(c == 0), stop=(c == 1))
                g2 = spool.tile([TOK, 256], f32, tag="g2")
                if "b_m2" in bias_on:
                    tb2 = spool.tile([TOK, 256], f32, tag="tb2")
                    nc.vector.tensor_add(out=tb2, in0=h2p[:, 0:256],
                                         in1=btiles["b_m2"][:TOK, :])
                    nc.scalar.activation(out=g2, in_=tb2,
                                         func=mybir.ActivationFunctionType.Gelu)
                else:
                    nc.scalar.activation(out=g2, in_=h2p[:, 0:256],
                                         func=mybir.ActivationFunctionType.Gelu)
                h2 = apool.tile([TOK, 256], f32, tag="h2")
                nc.gpsimd.tensor_add(out=h2, in0=g2, in1=h1)
                h2F = apool.tile([128, 2, TOK], f32r, tag="h2F")
                tp = ps_tr.tile([128, 4, TOK], f32, tag="tr")
                for c in range(2):
                    nc.tensor.transpose(tp[:, c + 2, :], h2[:, c * 128:(c + 1) * 128], ident)
                nc.vector.tensor_copy(out=h2F, in_=tp[:, 2:4, :])

                h3p = ps_one.tile([TOK, C], f32, tag="one")
                for c in range(2):
                    nc.tensor.matmul(h3p, h2F[:, c, :], m3s[:, c, :],
                                     start=(c == 0), stop=(c == 1))
                outT = apool.tile([TOK, C], f32, tag="outT")
                if "b_m3" in bias_on:
                    tb3 = spool.tile([TOK, C], f32, tag="tb3")
                    nc.vector.tensor_add(out=tb3, in0=h3p,
                                         in1=btiles["b_m3"][:TOK, :])
                    nc.scalar.activation(out=tb3, in_=tb3,
                                         func=mybir.ActivationFunctionType.Gelu)
                    nc.vector.tensor_add(out=outT, in0=tb3, in1=yT)
                else:
                    g3 = spool.tile([TOK, C], f32, tag="g3")
                    nc.scalar.activation(out=g3, in_=h3p,
                                         func=mybir.ActivationFunctionType.Gelu)
                    nc.gpsimd.tensor_add(out=outT, in0=g3, in1=yT)

                nc.sync.dma_start(out=outa[t0 + woff:t0 + woff + wntok, :],
                                  in_=outT[woff:woff + wntok, :])

    nc.compile()
    return nc


def _is_ones(a):
    return bool(np.all(a == 1.0))


def _is_zeros(a):
    return bool(np.all(a == 0.0))


def _prep(inputs):
    """Host-side folds and layout transforms. Returns (flags, shared arrays)."""
    adj = inputs["adj"].astype(np.float32)
    f64 = np.float64

    lnA_g, lnA_b = inputs["lnA_g"], inputs["lnA_b"]
    qkv_w = inputs["qkv_w"].astype(f64)
    wqkv = (qkv_w * lnA_g.astype(f64)[None, :])
    bqkv = inputs["qkv_b"].astype(f64) + qkv_w @ lnA_b.astype(f64)

    ln2_g, ln2_b = inputs["ln2_g"], inputs["ln2_b"]
    m1_w = inputs["m1_w"].astype(f64)
    wm1 = m1_w * ln2_g.astype(f64)[None, :]
    bm1 = inputs["m1_b"].astype(f64) + m1_w @ ln2_b.astype(f64)

    flags = set()
    if not (_is_ones(inputs["ln1_g"]) and _is_zeros(inputs["ln1_b"])):
        flags.add("ln1aff")
    shared = {
        "w1T": np.ascontiguousarray(inputs["gcn1_w"].astype(np.float32).T),
        "wqkvT": np.ascontiguousarray(wqkv.astype(np.float32).T),
        "wpT": np.ascontiguousarray(inputs["proj_w"].astype(np.float32).T),
        "w2T": np.ascontiguousarray(inputs["gcn2_w"].astype(np.float32).T),
        "m1T": np.ascontiguousarray(wm1.astype(np.float32).T),
        "m2T": np.ascontiguousarray(inputs["m2_w"].astype(np.float32).T),
        "m3T": np.ascontiguousarray(inputs["m3_w"].astype(np.float32).T),
        "g1d": inputs["ln1_g"].astype(np.float32),
        "b1d": inputs["ln1_b"].astype(np.float32),
    }
    ablk = np.zeros((TOK, K, TOK), np.float32)
    for k in range(K):
        for b in range(NB):
            ablk[b * J:(b + 1) * J, k, b * J:(b + 1) * J] = adj[k]
    shared["ablk"] = ablk.reshape(TOK, K * TOK)
    m = np.zeros((TOK, TOK), np.float32)
    for b in range(NB):
        m[b * J:(b + 1) * J, b * J:(b + 1) * J] = 1.0
    shared["maskd"] = m
    shared["identd"] = np.eye(TOK, dtype=np.float32)

    for nm, arr in [("b_qkv", bqkv.astype(np.float32)),
                    ("b_p", inputs["proj_b"].astype(np.float32)),
                    ("b_1g", inputs["gcn1_b"].astype(np.float32)),
                    ("b_2g", inputs["gcn2_b"].astype(np.float32)),
                    ("b_m1", bm1.astype(np.float32)),
                    ("b_m2", inputs["m2_b"].astype(np.float32)),
                    ("b_m3", inputs["m3_b"].astype(np.float32))]:
        if not _is_zeros(arr):
            flags.add(nm)
            shared[nm] = arr
    return frozenset(flags), shared


def kernel(**inputs):
    flags, shared = _prep(inputs)
    key = (B_CORE, flags)
    if key not in _CACHE:
        _CACHE[key] = _build(B_CORE, flags)
    nc = _CACHE[key]

    x = np.ascontiguousarray(inputs["x"], dtype=np.float32)
    in_maps = []
    for c in range(N_CORES):
        x2d = x[c * B_CORE:(c + 1) * B_CORE].reshape(B_CORE * J, C)
        m = dict(shared)
        m["x2d"] = x2d
        m["xTd"] = np.ascontiguousarray(x2d.T)
        in_maps.append(m)

    res = bass_utils.run_bass_kernel_spmd(nc, in_maps, core_ids=list(range(N_CORES)))
    outs = [res.results[c]["out"].reshape(B_CORE, J, C) for c in range(N_CORES)]
    return np.concatenate(outs, axis=0)

